# revision 55
# baseline (speedup 1.0000x reference)
"""LIF neuron (STBP) forward kernel for Trainium2, 8-core data parallel.

Reference semantics (per element, scan over T):
    v = v * 0.9 + x_t
    s = (v >= 1.0)
    v = v - s * 1.0

Sharding: batch dim 32 -> 8 cores x 4; the recurrence is elementwise per
neuron so cores are independent.

Layout: per core the input is relayouted on host to partition-major
[P=128, T*2048] f32; the whole 16-MiB input lives in one SBUF arena.
The kernel is HBM-load-bound (16 MiB of f32 input at ~350 GB/s/core),
so the design keeps every engine's busy time under the ~48 us load
window and shrinks store traffic 8x by bit-packing spikes on chip:

  Vector (DVE):   ONLY the serial LIF recurrence, one fused custom op
                  per step:  u' = (u - (u >= th)) * beta + x
  Scalar (Act):   spike extraction  s_t = sign(u_t - 1) in {-1,0,+1}
                  as bf16 (plus the two final pack copies psum->u8)
  Tensor (PE):    bit packing: psum += diag(2^(t%8)) @ s_t, so after 8
                  steps psum = sum_j +-2^j = 2*bits - 255; the Act copy
                  0.5*psum + 127.5 -> u8 yields the packed spike byte
  Sync/Act HWDGE: the 16 input loads (t14/t15 split small so the last
                  bytes land early); the 4 tail u8 stores
  GpSimd SWDGE:   the mid-kernel plane-A store (hidden under loads)

Host decodes spikes with unpackbits (bit j of plane g = spike at
t = 8g + j); sign(0)=0 at the measure-zero tie u == 1 is absorbed by
the rel-err budget.
"""

from contextlib import ExitStack

import numpy as np

import concourse.bacc as bacc
import concourse.mybir as mybir
import concourse.tile as tile
from concourse.bass_utils import run_bass_kernel_spmd

N_CORES = 8
B, T, C, H, W = 32, 16, 64, 32, 32
B_LOC = B // N_CORES  # 4 batches per core
P = 128               # SBUF partitions
F = (C * H * W) // P  # 512 free elements per partition per batch
FB = B_LOC * F        # 2048 free elements in a fused all-batch tile
Q = FB // 4           # 512 = max matmul moving free dim
BETA = 0.9
V_TH = 1.0

_CACHE = {}


def _get_lif_op():
    """Register (once) and return the fused LIF membrane-update DVE op."""
    import concourse.dve_ops as dve_ops
    from concourse.dve_ops import DveOp
    from concourse.dve_spec import C0, C1, Spec, Src0, Src1

    for o in dve_ops.OPS:
        if o.name == "LIF_U_ANT":
            return o

    op = DveOp(
        "LIF_U_ANT",
        Spec(
            body=(Src0 - (Src0 >= C1)) * C0 + Src1,
            reference=lambda in0, in1, s0, s1, imm2: (
                ((in0 - (in0 >= np.float32(s1)).astype(np.float32))
                 .astype(np.float32) * np.float32(s0) + in1).astype(np.float32)
            ),
        ),
        subdim=False,
        uops_sha={"v3": "5dffcaa405b6c09a", "v4": "7706b30f0e4fb094"},
    )
    dve_ops.OPS.append(op)
    dve_ops.CUSTOM_DVE_SPECS[op.name] = op.spec
    dve_ops._SUB_OPCODE_FOR_NAME[op.name] = (
        dve_ops._CUSTOM_DVE_ROW_BASE + len(dve_ops.OPS) - 1
    )
    return op


def _build(repeat: int = 1):
    lif_u = _get_lif_op()
    nc = bacc.Bacc(
        "TRN2", target_bir_lowering=False, debug=False, num_devices=N_CORES
    )
    x = nc.dram_tensor(
        "x", [P, T * FB], mybir.dt.float32, kind="ExternalInput"
    ).ap()
    s_out = nc.dram_tensor(
        "s", [P, 2 * FB], mybir.dt.uint8, kind="ExternalOutput"
    ).ap()

    # Const diag-block pack weights, embedded in the NEFF. Block j
    # (j=0..7) = +2^j * I for {0,1} is_ge-coded spike tiles (vector,
    # t0..t11). Blocks 8+i (i=0..3) = -2^(j-1) * I for the sign-coded
    # tail tiles (Act, t=12..15, j=4..7): contribution 2^j*s - 2^(j-1),
    # the offsets sum to 120 and fold into plane B's pack bias.
    wnp = np.zeros((P, 16 * 128), dtype=np.float32)
    for j in range(8):
        for p in range(P):
            wnp[p, j * 128 + p] = 2.0 ** j
            wnp[p, (8 + j) * 128 + p] = -(2.0 ** j) / 2.0
    wdram = nc.inline_tensor(
        wnp.astype(mybir.dt.np(mybir.dt.float8e4)), name="wconst"
    )

    with tile.TileContext(nc) as tc:
        _emit(nc, tc, x, s_out, repeat, lif_u, wdram)

    nc.compile()
    return nc


def _emit(nc, tc, x, s_out, repeat, lif_u, wdram):
    Sign = mybir.ActivationFunctionType.Sign
    Copy = mybir.ActivationFunctionType.Copy
    h = FB // 2

    def xsl(t, a=0, b=FB):
        return slice(t * FB + a, t * FB + b)

    with ExitStack() as ctx:
        xp = ctx.enter_context(tc.tile_pool(name="xp", bufs=1))
        up = ctx.enter_context(tc.tile_pool(name="up", bufs=5))
        sp = ctx.enter_context(tc.tile_pool(name="sp", bufs=4))
        wp = ctx.enter_context(tc.tile_pool(name="wp", bufs=2))
        op = ctx.enter_context(tc.tile_pool(name="op", bufs=1))
        pp = ctx.enter_context(tc.tile_pool(name="pp", bufs=1, space="PSUM"))

        qs = [nc.sync, nc.scalar]

        for _ in range(repeat):
            xall = xp.tile([P, T * FB], mybir.dt.float32)

            # --- input loads: half-column transfers (512 KiB, 4-KiB
            # rows) spread over THREE queues -- sync HWDGE, scalar HWDGE
            # and the gpsimd SWDGE queue -- in proportion to their
            # measured service rates under concurrent compute
            # (~209:104:65 GB/s). Sequential assignment keeps arrival
            # order == consumption order within each queue.
            # 3-queue split tuned to measured per-queue service rates
            # (sync ~240, scalar ~95, SWDGE ~55 GB/s): sync carries 21
            # halves, scalar 8 (later-deadline steps -- it delivers one
            # half per ~5.4 us), SWDGE 3 mid-kernel halves. Sequential
            # per-queue order preserves arrival==consumption order.
            SCALAR_HALVES = set()
            SW_HALVES = {(1, 0), (2, 0), (3, 0), (4, 0), (5, 0),
                         (6, 0), (7, 0)}
            for t in range(T - 2):
                for k in (0, 1):
                    a, b = k * h, (k + 1) * h
                    if (t, k) in SW_HALVES:
                        eng = nc.gpsimd
                    elif (t, k) in SCALAR_HALVES:
                        eng = nc.scalar
                    else:
                        eng = nc.sync
                    eng.dma_start(xall[:, xsl(t, a, b)], x[:, xsl(t, a, b)])
            for t in (14, 15):
                for kq in range(4):
                    a, b = kq * Q, (kq + 1) * Q
                    nc.sync.dma_start(xall[:, xsl(t, a, b)], x[:, xsl(t, a, b)])

            # --- pack weights: one SWDGE load of the NEFF-const diag
            # blocks (keeps both HWDGE queues free for input).
            w = wp.tile([P, 16 * 128], mybir.dt.float8e4, bufs=1)
            nc.scalar.dma_start(w[:], wdram.ap())

            psum = pp.tile([P, 2 * FB], mybir.dt.float32)   # all 8 banks
            outp = op.tile([P, 2 * FB], mybir.dt.uint8)

            def lif(out_ap, in0_ap, in1_ap):
                nc.vector._custom_dve(
                    lif_u, out=out_ap, in0=in0_ap, in1=in1_ap,
                    s0=BETA, s1=V_TH,
                )

            def sgn2(out_ap, in_ap):
                # Act-coded extract: sign(1 - u) in {+1, 0, -1}.
                # (bias must be a pre-registered const AP; 1.0 exists.)
                nc.scalar.activation(out_ap, in_ap, Sign, bias=V_TH, scale=-1.0)

            def isge(eng, out_ap, in_ap):
                # {0,1}-coded extract on vector (2x mode) or gpsimd
                eng.tensor_scalar(
                    out_ap, in_ap, V_TH, None, mybir.AluOpType.is_ge
                )

            def mm(t, s_t, a, b):
                g, j = t // 8, t % 8
                blk = j if t >= 14 else 8 + j
                for qa in range(a, b, Q):
                    nc.tensor.matmul(
                        psum[:, g * FB + qa: g * FB + qa + Q],
                        lhsT=w[:, blk * 128:(blk + 1) * 128],
                        rhs=s_t[:, qa:qa + Q],
                        start=(j == 0), stop=(j == 7),
                    )

            def pack(a, b, bias):
                # packed byte = psum + bias (exact integer in f32).
                # gpsimd can't read PSUM, so this lives on Act.
                nc.scalar.activation(
                    outp[:, a:b], psum[:, a:b], Copy, bias=bias, scale=1.0,
                )

            # Spike extraction: ALL on vector (is_ge, {0,1}-coded, 2x
            # DVE mode). Compute on the HWDGE trigger engines (sync,
            # scalar/Act) halves their DMA queue throughput, so Act only
            # triggers loads and runs the 5 pack copies (packA is the one
            # brief mid-load op; packB runs after the loads drain).
            # gpsimd tensor ops are ~12-50x below roofline; never put
            # compute there.
            def extract(t, out_ap, in_ap):
                # Keep the serial vector chain short: vector only extracts
                # the first half (is_ge, {0,1}); Act extracts t8..t15
                # (sign, +-1-coded) in parallel with the LIF chain.
                if t >= 14:
                    isge(nc.vector, out_ap, in_ap)
                else:
                    sgn2(out_ap, in_ap)

            # t0: u0 = x0, no LIF op
            s0 = sp.tile([P, FB], mybir.dt.float8e4, name="s", tag="s")
            extract(0, s0[:], xall[:, xsl(0)])
            mm(0, s0[:], 0, FB)
            u_prev = xall[:, xsl(0)]

            for t in range(1, 14):
                un = up.tile([P, FB], mybir.dt.float32, name="u", tag="u")
                lif(un[:], u_prev, xall[:, xsl(t)])
                st = sp.tile([P, FB], mybir.dt.float8e4, name="s", tag="s")
                extract(t, st[:], un[:])
                mm(t, st[:], 0, FB)
                u_prev = un[:]
                if t == 7:
                    # plane A complete: pack + SWDGE store (hidden under
                    # the ongoing loads)
                    pack(0, FB, 127.5)
                    nc.gpsimd.dma_start(s_out[:, 0:FB], outp[:, 0:FB])

            # t14 in quarters (matches its quartered loads; lets each
            # t15 column slice start as soon as its t14 quarter is done)
            u14 = up.tile([P, FB], mybir.dt.float32, name="u", tag="u")
            s14 = sp.tile([P, FB], mybir.dt.float8e4, name="s", tag="s")
            for kq in range(4):
                sl = slice(kq * Q, (kq + 1) * Q)
                lif(u14[:, sl], u_prev[:, sl], xall[:, xsl(14, sl.start, sl.stop)])
                extract(14, s14[:, sl], u14[:, sl])
                mm(14, s14[:], sl.start, sl.stop)

            # t15 in quarters: lif -> extract -> matmul(stop) -> pack ->
            # store, stores on the by-now-idle sync HWDGE queue
            u15 = up.tile([P, FB], mybir.dt.float32, name="u", tag="u")
            s15 = sp.tile([P, FB], mybir.dt.float8e4, name="s", tag="s")
            for k in range(4):
                sl = slice(k * Q, (k + 1) * Q)
                lif(u15[:, sl], u14[:, sl], xall[:, xsl(15, sl.start, sl.stop)])
                extract(15, s15[:, sl], u15[:, sl])
                mm(15, s15[:], sl.start, sl.stop)
                pack(FB + sl.start, FB + sl.stop, 31.5)
                nc.sync.dma_start(
                    s_out[:, FB + sl.start:FB + sl.stop],
                    outp[:, FB + sl.start:FB + sl.stop],
                )


def _get_nc(repeat: int = 1):
    key = f"nc{repeat}"
    if key not in _CACHE:
        _CACHE[key] = _build(repeat)
    return _CACHE[key]


def _shard_input(x_seq: np.ndarray, i: int) -> np.ndarray:
    # [4, T, C, H, W] -> partition-major arena layout [P, T*B_LOC*F].
    xc = x_seq[i * B_LOC:(i + 1) * B_LOC].reshape(B_LOC, T, P, F)
    return np.ascontiguousarray(xc.transpose(2, 1, 0, 3).reshape(P, T * FB))


def _unshard_output(s_u8: np.ndarray) -> np.ndarray:
    # [P, 2*FB] u8 -> [B_LOC, T, C, H, W] f32 spikes.
    # Plane g in {0,1} holds bit j = spike at t = 8g + j.
    pl = s_u8.reshape(P, 2, B_LOC, F)
    bits = np.unpackbits(pl[..., None], axis=-1, bitorder="little")
    # dims (P, g, b, F, j) -> (b, g, j, P, F) -> [B_LOC, T, P, F]
    s = bits.transpose(2, 1, 4, 0, 3).reshape(B_LOC, T, P, F)
    return s.astype(np.float32).reshape(B_LOC, T, C, H, W)


def _run(x_seq: np.ndarray, trace: bool = False, repeat: int = 1):
    """Shard, execute on 8 cores, gather. Returns (output, BassKernelResults)."""
    nc = _get_nc(repeat)
    x_seq = np.ascontiguousarray(x_seq, dtype=np.float32)
    in_maps = [{"x": _shard_input(x_seq, i)} for i in range(N_CORES)]
    res = run_bass_kernel_spmd(
        nc, in_maps, core_ids=list(range(N_CORES)), trace=trace
    )
    out = np.concatenate(
        [_unshard_output(r["s"]) for r in res.results], axis=0
    )
    return out, res


def kernel(x_seq: np.ndarray) -> np.ndarray:
    out, _ = _run(x_seq, trace=False)
    return out


# revision 56
# speedup vs baseline: 1.0098x; 1.0098x over previous
"""LIF neuron (STBP) forward kernel for Trainium2, 8-core data parallel.

Reference semantics (per element, scan over T):
    v = v * 0.9 + x_t
    s = (v >= 1.0)
    v = v - s * 1.0

Sharding: batch dim 32 -> 8 cores x 4; the recurrence is elementwise per
neuron so cores are independent.

Layout: per core the input is relayouted on host to partition-major
[P=128, T*2048] f32; the whole 16-MiB input lives in one SBUF arena.
The kernel is HBM-load-bound (16 MiB of f32 input at ~350 GB/s/core),
so the design keeps every engine's busy time under the ~48 us load
window and shrinks store traffic 8x by bit-packing spikes on chip:

  Vector (DVE):   ONLY the serial LIF recurrence, one fused custom op
                  per step:  u' = (u - (u >= th)) * beta + x
  Scalar (Act):   spike extraction  s_t = sign(u_t - 1) in {-1,0,+1}
                  as bf16 (plus the two final pack copies psum->u8)
  Tensor (PE):    bit packing: psum += diag(2^(t%8)) @ s_t, so after 8
                  steps psum = sum_j +-2^j = 2*bits - 255; the Act copy
                  0.5*psum + 127.5 -> u8 yields the packed spike byte
  Sync/Act HWDGE: the 16 input loads (t14/t15 split small so the last
                  bytes land early); the 4 tail u8 stores
  GpSimd SWDGE:   the mid-kernel plane-A store (hidden under loads)

Host decodes spikes with unpackbits (bit j of plane g = spike at
t = 8g + j); sign(0)=0 at the measure-zero tie u == 1 is absorbed by
the rel-err budget.
"""

from contextlib import ExitStack

import numpy as np

import concourse.bacc as bacc
import concourse.mybir as mybir
import concourse.tile as tile
from concourse.bass_utils import run_bass_kernel_spmd

N_CORES = 8
B, T, C, H, W = 32, 16, 64, 32, 32
B_LOC = B // N_CORES  # 4 batches per core
P = 128               # SBUF partitions
F = (C * H * W) // P  # 512 free elements per partition per batch
FB = B_LOC * F        # 2048 free elements in a fused all-batch tile
Q = FB // 4           # 512 = max matmul moving free dim
BETA = 0.9
V_TH = 1.0

_CACHE = {}


def _get_lif_op():
    """Register (once) and return the fused LIF membrane-update DVE op."""
    import concourse.dve_ops as dve_ops
    from concourse.dve_ops import DveOp
    from concourse.dve_spec import C0, C1, Spec, Src0, Src1

    for o in dve_ops.OPS:
        if o.name == "LIF_U_ANT":
            return o

    op = DveOp(
        "LIF_U_ANT",
        Spec(
            body=(Src0 - (Src0 >= C1)) * C0 + Src1,
            reference=lambda in0, in1, s0, s1, imm2: (
                ((in0 - (in0 >= np.float32(s1)).astype(np.float32))
                 .astype(np.float32) * np.float32(s0) + in1).astype(np.float32)
            ),
        ),
        subdim=False,
        uops_sha={"v3": "5dffcaa405b6c09a", "v4": "7706b30f0e4fb094"},
    )
    dve_ops.OPS.append(op)
    dve_ops.CUSTOM_DVE_SPECS[op.name] = op.spec
    dve_ops._SUB_OPCODE_FOR_NAME[op.name] = (
        dve_ops._CUSTOM_DVE_ROW_BASE + len(dve_ops.OPS) - 1
    )
    return op


def _build(repeat: int = 1):
    lif_u = _get_lif_op()
    nc = bacc.Bacc(
        "TRN2", target_bir_lowering=False, debug=False, num_devices=N_CORES
    )
    x = nc.dram_tensor(
        "x", [P, T * FB], mybir.dt.float32, kind="ExternalInput"
    ).ap()
    s_out = nc.dram_tensor(
        "s", [P, 2 * FB], mybir.dt.uint8, kind="ExternalOutput"
    ).ap()

    # Const diag-block pack weights, embedded in the NEFF. Block j
    # (j=0..7) = +2^j * I for {0,1} is_ge-coded spike tiles (vector,
    # t0..t11). Blocks 8+i (i=0..3) = -2^(j-1) * I for the sign-coded
    # tail tiles (Act, t=12..15, j=4..7): contribution 2^j*s - 2^(j-1),
    # the offsets sum to 120 and fold into plane B's pack bias.
    wnp = np.zeros((P, 16 * 128), dtype=np.float32)
    for j in range(8):
        for p in range(P):
            wnp[p, j * 128 + p] = 2.0 ** j
            wnp[p, (8 + j) * 128 + p] = -(2.0 ** j) / 2.0
    wdram = nc.inline_tensor(
        wnp.astype(mybir.dt.np(mybir.dt.float8e4)), name="wconst"
    )

    with tile.TileContext(nc) as tc:
        _emit(nc, tc, x, s_out, repeat, lif_u, wdram)

    nc.compile()
    return nc


def _emit(nc, tc, x, s_out, repeat, lif_u, wdram):
    Sign = mybir.ActivationFunctionType.Sign
    Copy = mybir.ActivationFunctionType.Copy
    h = FB // 2

    def xsl(t, a=0, b=FB):
        return slice(t * FB + a, t * FB + b)

    with ExitStack() as ctx:
        xp = ctx.enter_context(tc.tile_pool(name="xp", bufs=1))
        up = ctx.enter_context(tc.tile_pool(name="up", bufs=5))
        sp = ctx.enter_context(tc.tile_pool(name="sp", bufs=4))
        wp = ctx.enter_context(tc.tile_pool(name="wp", bufs=2))
        op = ctx.enter_context(tc.tile_pool(name="op", bufs=1))
        pp = ctx.enter_context(tc.tile_pool(name="pp", bufs=1, space="PSUM"))

        qs = [nc.sync, nc.scalar]

        for _ in range(repeat):
            xall = xp.tile([P, T * FB], mybir.dt.float32)

            # --- input loads: half-column transfers (512 KiB, 4-KiB
            # rows) spread over THREE queues -- sync HWDGE, scalar HWDGE
            # and the gpsimd SWDGE queue -- in proportion to their
            # measured service rates under concurrent compute
            # (~209:104:65 GB/s). Sequential assignment keeps arrival
            # order == consumption order within each queue.
            # 3-queue split tuned to measured per-queue service rates
            # (sync ~240, scalar ~95, SWDGE ~55 GB/s): sync carries 21
            # halves, scalar 8 (later-deadline steps -- it delivers one
            # half per ~5.4 us), SWDGE 3 mid-kernel halves. Sequential
            # per-queue order preserves arrival==consumption order.
            SCALAR_HALVES = set()
            SW_HALVES = {(1, 0), (2, 0), (3, 0), (4, 0), (5, 0),
                         (6, 0), (7, 0)}
            for t in range(T - 3):
                for k in (0, 1):
                    a, b = k * h, (k + 1) * h
                    if (t, k) in SW_HALVES:
                        eng = nc.gpsimd
                    elif (t, k) in SCALAR_HALVES:
                        eng = nc.scalar
                    else:
                        eng = nc.sync
                    eng.dma_start(xall[:, xsl(t, a, b)], x[:, xsl(t, a, b)])
            for t in (13, 14, 15):
                for kq in range(4):
                    a, b = kq * Q, (kq + 1) * Q
                    nc.sync.dma_start(xall[:, xsl(t, a, b)], x[:, xsl(t, a, b)])

            # --- pack weights: one SWDGE load of the NEFF-const diag
            # blocks (keeps both HWDGE queues free for input).
            w = wp.tile([P, 16 * 128], mybir.dt.float8e4, bufs=1)
            nc.scalar.dma_start(w[:], wdram.ap())

            psum = pp.tile([P, 2 * FB], mybir.dt.float32)   # all 8 banks
            outp = op.tile([P, 2 * FB], mybir.dt.uint8)

            def lif(out_ap, in0_ap, in1_ap):
                nc.vector._custom_dve(
                    lif_u, out=out_ap, in0=in0_ap, in1=in1_ap,
                    s0=BETA, s1=V_TH,
                )

            def sgn2(out_ap, in_ap):
                # Act-coded extract: sign(1 - u) in {+1, 0, -1}.
                # (bias must be a pre-registered const AP; 1.0 exists.)
                nc.scalar.activation(out_ap, in_ap, Sign, bias=V_TH, scale=-1.0)

            def isge(eng, out_ap, in_ap):
                # {0,1}-coded extract on vector (2x mode) or gpsimd
                eng.tensor_scalar(
                    out_ap, in_ap, V_TH, None, mybir.AluOpType.is_ge
                )

            def mm(t, s_t, a, b):
                g, j = t // 8, t % 8
                blk = j if t >= 14 else 8 + j
                for qa in range(a, b, Q):
                    nc.tensor.matmul(
                        psum[:, g * FB + qa: g * FB + qa + Q],
                        lhsT=w[:, blk * 128:(blk + 1) * 128],
                        rhs=s_t[:, qa:qa + Q],
                        start=(j == 0), stop=(j == 7),
                    )

            def pack(a, b, bias):
                # packed byte = psum + bias (exact integer in f32).
                # gpsimd can't read PSUM, so this lives on Act.
                nc.scalar.activation(
                    outp[:, a:b], psum[:, a:b], Copy, bias=bias, scale=1.0,
                )

            # Spike extraction: ALL on vector (is_ge, {0,1}-coded, 2x
            # DVE mode). Compute on the HWDGE trigger engines (sync,
            # scalar/Act) halves their DMA queue throughput, so Act only
            # triggers loads and runs the 5 pack copies (packA is the one
            # brief mid-load op; packB runs after the loads drain).
            # gpsimd tensor ops are ~12-50x below roofline; never put
            # compute there.
            def extract(t, out_ap, in_ap):
                # Keep the serial vector chain short: vector only extracts
                # the first half (is_ge, {0,1}); Act extracts t8..t15
                # (sign, +-1-coded) in parallel with the LIF chain.
                if t >= 14:
                    isge(nc.vector, out_ap, in_ap)
                else:
                    sgn2(out_ap, in_ap)

            # t0: u0 = x0, no LIF op
            s0 = sp.tile([P, FB], mybir.dt.float8e4, name="s", tag="s")
            extract(0, s0[:], xall[:, xsl(0)])
            mm(0, s0[:], 0, FB)
            u_prev = xall[:, xsl(0)]

            for t in range(1, 13):
                un = up.tile([P, FB], mybir.dt.float32, name="u", tag="u")
                lif(un[:], u_prev, xall[:, xsl(t)])
                st = sp.tile([P, FB], mybir.dt.float8e4, name="s", tag="s")
                extract(t, st[:], un[:])
                mm(t, st[:], 0, FB)
                u_prev = un[:]
                if t == 7:
                    # plane A complete: pack + SWDGE store (hidden under
                    # the ongoing loads)
                    pack(0, FB, 127.5)
                    nc.gpsimd.dma_start(s_out[:, 0:FB], outp[:, 0:FB])

            # t13 in quarters (same rationale as t14/t15 below)
            u13 = up.tile([P, FB], mybir.dt.float32, name="u", tag="u")
            s13 = sp.tile([P, FB], mybir.dt.float8e4, name="s", tag="s")
            for kq in range(4):
                sl = slice(kq * Q, (kq + 1) * Q)
                lif(u13[:, sl], u_prev[:, sl], xall[:, xsl(13, sl.start, sl.stop)])
                extract(13, s13[:, sl], u13[:, sl])
                mm(13, s13[:], sl.start, sl.stop)
            u_prev = u13[:]

            # t14 in quarters (matches its quartered loads; lets each
            # t15 column slice start as soon as its t14 quarter is done)
            u14 = up.tile([P, FB], mybir.dt.float32, name="u", tag="u")
            s14 = sp.tile([P, FB], mybir.dt.float8e4, name="s", tag="s")
            for kq in range(4):
                sl = slice(kq * Q, (kq + 1) * Q)
                lif(u14[:, sl], u_prev[:, sl], xall[:, xsl(14, sl.start, sl.stop)])
                extract(14, s14[:, sl], u14[:, sl])
                mm(14, s14[:], sl.start, sl.stop)

            # t15 in quarters: lif -> extract -> matmul(stop) -> pack ->
            # store, stores on the by-now-idle sync HWDGE queue
            u15 = up.tile([P, FB], mybir.dt.float32, name="u", tag="u")
            s15 = sp.tile([P, FB], mybir.dt.float8e4, name="s", tag="s")
            for k in range(4):
                sl = slice(k * Q, (k + 1) * Q)
                lif(u15[:, sl], u14[:, sl], xall[:, xsl(15, sl.start, sl.stop)])
                extract(15, s15[:, sl], u15[:, sl])
                mm(15, s15[:], sl.start, sl.stop)
                pack(FB + sl.start, FB + sl.stop, 31.5)
                nc.sync.dma_start(
                    s_out[:, FB + sl.start:FB + sl.stop],
                    outp[:, FB + sl.start:FB + sl.stop],
                )


def _get_nc(repeat: int = 1):
    key = f"nc{repeat}"
    if key not in _CACHE:
        _CACHE[key] = _build(repeat)
    return _CACHE[key]


def _shard_input(x_seq: np.ndarray, i: int) -> np.ndarray:
    # [4, T, C, H, W] -> partition-major arena layout [P, T*B_LOC*F].
    xc = x_seq[i * B_LOC:(i + 1) * B_LOC].reshape(B_LOC, T, P, F)
    return np.ascontiguousarray(xc.transpose(2, 1, 0, 3).reshape(P, T * FB))


def _unshard_output(s_u8: np.ndarray) -> np.ndarray:
    # [P, 2*FB] u8 -> [B_LOC, T, C, H, W] f32 spikes.
    # Plane g in {0,1} holds bit j = spike at t = 8g + j.
    pl = s_u8.reshape(P, 2, B_LOC, F)
    bits = np.unpackbits(pl[..., None], axis=-1, bitorder="little")
    # dims (P, g, b, F, j) -> (b, g, j, P, F) -> [B_LOC, T, P, F]
    s = bits.transpose(2, 1, 4, 0, 3).reshape(B_LOC, T, P, F)
    return s.astype(np.float32).reshape(B_LOC, T, C, H, W)


def _run(x_seq: np.ndarray, trace: bool = False, repeat: int = 1):
    """Shard, execute on 8 cores, gather. Returns (output, BassKernelResults)."""
    nc = _get_nc(repeat)
    x_seq = np.ascontiguousarray(x_seq, dtype=np.float32)
    in_maps = [{"x": _shard_input(x_seq, i)} for i in range(N_CORES)]
    res = run_bass_kernel_spmd(
        nc, in_maps, core_ids=list(range(N_CORES)), trace=trace
    )
    out = np.concatenate(
        [_unshard_output(r["s"]) for r in res.results], axis=0
    )
    return out, res


def kernel(x_seq: np.ndarray) -> np.ndarray:
    out, _ = _run(x_seq, trace=False)
    return out


# revision 57
# speedup vs baseline: 1.0797x; 1.0692x over previous
"""LIF neuron (STBP) forward kernel for Trainium2, 8-core data parallel.

Reference semantics (per element, scan over T):
    v = v * 0.9 + x_t
    s = (v >= 1.0)
    v = v - s * 1.0

Sharding: batch dim 32 -> 8 cores x 4; the recurrence is elementwise per
neuron so cores are independent.

Layout: per core the input is relayouted on host to partition-major
[P=128, T*2048] f32; the whole 16-MiB input lives in one SBUF arena.
The kernel is HBM-load-bound (16 MiB of f32 input at ~350 GB/s/core),
so the design keeps every engine's busy time under the ~48 us load
window and shrinks store traffic 8x by bit-packing spikes on chip:

  Vector (DVE):   ONLY the serial LIF recurrence, one fused custom op
                  per step:  u' = (u - (u >= th)) * beta + x
  Scalar (Act):   spike extraction  s_t = sign(u_t - 1) in {-1,0,+1}
                  as bf16 (plus the two final pack copies psum->u8)
  Tensor (PE):    bit packing: psum += diag(2^(t%8)) @ s_t, so after 8
                  steps psum = sum_j +-2^j = 2*bits - 255; the Act copy
                  0.5*psum + 127.5 -> u8 yields the packed spike byte
  Sync/Act HWDGE: the 16 input loads (t14/t15 split small so the last
                  bytes land early); the 4 tail u8 stores
  GpSimd SWDGE:   the mid-kernel plane-A store (hidden under loads)

Host decodes spikes with unpackbits (bit j of plane g = spike at
t = 8g + j); sign(0)=0 at the measure-zero tie u == 1 is absorbed by
the rel-err budget.
"""

from contextlib import ExitStack

import numpy as np

import concourse.bacc as bacc
import concourse.mybir as mybir
import concourse.tile as tile
from concourse.bass_utils import run_bass_kernel_spmd

N_CORES = 8
B, T, C, H, W = 32, 16, 64, 32, 32
B_LOC = B // N_CORES  # 4 batches per core
P = 128               # SBUF partitions
F = (C * H * W) // P  # 512 free elements per partition per batch
FB = B_LOC * F        # 2048 free elements in a fused all-batch tile
Q = FB // 4           # 512 = max matmul moving free dim
BETA = 0.9
V_TH = 1.0

_CACHE = {}


def _get_lif_op():
    """Register (once) and return the fused LIF membrane-update DVE op."""
    import concourse.dve_ops as dve_ops
    from concourse.dve_ops import DveOp
    from concourse.dve_spec import C0, C1, Spec, Src0, Src1

    for o in dve_ops.OPS:
        if o.name == "LIF_U_ANT":
            return o

    op = DveOp(
        "LIF_U_ANT",
        Spec(
            body=(Src0 - (Src0 >= C1)) * C0 + Src1,
            reference=lambda in0, in1, s0, s1, imm2: (
                ((in0 - (in0 >= np.float32(s1)).astype(np.float32))
                 .astype(np.float32) * np.float32(s0) + in1).astype(np.float32)
            ),
        ),
        subdim=False,
        uops_sha={"v3": "5dffcaa405b6c09a", "v4": "7706b30f0e4fb094"},
    )
    dve_ops.OPS.append(op)
    dve_ops.CUSTOM_DVE_SPECS[op.name] = op.spec
    dve_ops._SUB_OPCODE_FOR_NAME[op.name] = (
        dve_ops._CUSTOM_DVE_ROW_BASE + len(dve_ops.OPS) - 1
    )
    return op


def _build(repeat: int = 1):
    lif_u = _get_lif_op()
    nc = bacc.Bacc(
        "TRN2", target_bir_lowering=False, debug=False, num_devices=N_CORES
    )
    x = nc.dram_tensor(
        "x", [P, T * FB], mybir.dt.float32, kind="ExternalInput"
    ).ap()
    s_out = nc.dram_tensor(
        "s", [P, 2 * FB], mybir.dt.uint8, kind="ExternalOutput"
    ).ap()

    # Const diag-block pack weights, embedded in the NEFF. Block j
    # (j=0..7) = +2^j * I for {0,1} is_ge-coded spike tiles (vector,
    # t0..t11). Blocks 8+i (i=0..3) = -2^(j-1) * I for the sign-coded
    # tail tiles (Act, t=12..15, j=4..7): contribution 2^j*s - 2^(j-1),
    # the offsets sum to 120 and fold into plane B's pack bias.
    wnp = np.zeros((P, 16 * 128), dtype=np.float32)
    for j in range(8):
        for p in range(P):
            wnp[p, j * 128 + p] = 2.0 ** j
            wnp[p, (8 + j) * 128 + p] = -(2.0 ** j) / 2.0
    wdram = nc.inline_tensor(
        wnp.astype(mybir.dt.np(mybir.dt.float8e4)), name="wconst"
    )

    with tile.TileContext(nc) as tc:
        _emit(nc, tc, x, s_out, repeat, lif_u, wdram)

    nc.compile()
    return nc


def _emit(nc, tc, x, s_out, repeat, lif_u, wdram):
    Sign = mybir.ActivationFunctionType.Sign
    Copy = mybir.ActivationFunctionType.Copy
    h = FB // 2

    def xsl(t, a=0, b=FB):
        return slice(t * FB + a, t * FB + b)

    with ExitStack() as ctx:
        xp = ctx.enter_context(tc.tile_pool(name="xp", bufs=1))
        up = ctx.enter_context(tc.tile_pool(name="up", bufs=5))
        sp = ctx.enter_context(tc.tile_pool(name="sp", bufs=4))
        wp = ctx.enter_context(tc.tile_pool(name="wp", bufs=2))
        op = ctx.enter_context(tc.tile_pool(name="op", bufs=1))
        pp = ctx.enter_context(tc.tile_pool(name="pp", bufs=1, space="PSUM"))

        qs = [nc.sync, nc.scalar]

        for _ in range(repeat):
            xall = xp.tile([P, T * FB], mybir.dt.float32)

            # --- input loads: half-column transfers (512 KiB, 4-KiB
            # rows) spread over THREE queues -- sync HWDGE, scalar HWDGE
            # and the gpsimd SWDGE queue -- in proportion to their
            # measured service rates under concurrent compute
            # (~209:104:65 GB/s). Sequential assignment keeps arrival
            # order == consumption order within each queue.
            # 3-queue split tuned to measured per-queue service rates
            # (sync ~240, scalar ~95, SWDGE ~55 GB/s): sync carries 21
            # halves, scalar 8 (later-deadline steps -- it delivers one
            # half per ~5.4 us), SWDGE 3 mid-kernel halves. Sequential
            # per-queue order preserves arrival==consumption order.
            SCALAR_HALVES = set()
            SW_HALVES = {(1, 0), (2, 0), (3, 0), (4, 0), (5, 0),
                         (6, 0), (7, 0)}
            for t in range(T - 2):
                for k in (0, 1):
                    a, b = k * h, (k + 1) * h
                    if (t, k) in SW_HALVES:
                        eng = nc.gpsimd
                    elif (t, k) in SCALAR_HALVES:
                        eng = nc.scalar
                    else:
                        eng = nc.sync
                    eng.dma_start(xall[:, xsl(t, a, b)], x[:, xsl(t, a, b)])
            for t in (14, 15):
                for kq in range(4):
                    a, b = kq * Q, (kq + 1) * Q
                    nc.sync.dma_start(xall[:, xsl(t, a, b)], x[:, xsl(t, a, b)])

            # --- pack weights: one SWDGE load of the NEFF-const diag
            # blocks (keeps both HWDGE queues free for input).
            w = wp.tile([P, 16 * 128], mybir.dt.float8e4, bufs=1)
            nc.scalar.dma_start(w[:], wdram.ap())

            psum = pp.tile([P, 2 * FB], mybir.dt.float32)   # all 8 banks
            outp = op.tile([P, 2 * FB], mybir.dt.uint8)

            def lif(out_ap, in0_ap, in1_ap):
                nc.vector._custom_dve(
                    lif_u, out=out_ap, in0=in0_ap, in1=in1_ap,
                    s0=BETA, s1=V_TH,
                )

            def sgn2(out_ap, in_ap):
                # Act-coded extract: sign(1 - u) in {+1, 0, -1}.
                # (bias must be a pre-registered const AP; 1.0 exists.)
                nc.scalar.activation(out_ap, in_ap, Sign, bias=V_TH, scale=-1.0)

            def isge(eng, out_ap, in_ap):
                # {0,1}-coded extract on vector (2x mode) or gpsimd
                eng.tensor_scalar(
                    out_ap, in_ap, V_TH, None, mybir.AluOpType.is_ge
                )

            def mm(t, s_t, a, b):
                g, j = t // 8, t % 8
                blk = j if t >= 14 else 8 + j
                for qa in range(a, b, Q):
                    nc.tensor.matmul(
                        psum[:, g * FB + qa: g * FB + qa + Q],
                        lhsT=w[:, blk * 128:(blk + 1) * 128],
                        rhs=s_t[:, qa:qa + Q],
                        start=(j == 0), stop=(j == 7),
                    )

            def pack(a, b, bias):
                # packed byte = psum + bias (exact integer in f32).
                # gpsimd can't read PSUM, so this lives on Act.
                nc.scalar.activation(
                    outp[:, a:b], psum[:, a:b], Copy, bias=bias, scale=1.0,
                )

            # Spike extraction: ALL on vector (is_ge, {0,1}-coded, 2x
            # DVE mode). Compute on the HWDGE trigger engines (sync,
            # scalar/Act) halves their DMA queue throughput, so Act only
            # triggers loads and runs the 5 pack copies (packA is the one
            # brief mid-load op; packB runs after the loads drain).
            # gpsimd tensor ops are ~12-50x below roofline; never put
            # compute there.
            def extract(t, out_ap, in_ap):
                # Keep the serial vector chain short: vector only extracts
                # the first half (is_ge, {0,1}); Act extracts t8..t15
                # (sign, +-1-coded) in parallel with the LIF chain.
                if t >= 14:
                    isge(nc.vector, out_ap, in_ap)
                else:
                    sgn2(out_ap, in_ap)

            # t0: u0 = x0, no LIF op
            s0 = sp.tile([P, FB], mybir.dt.float8e4, name="s", tag="s")
            extract(0, s0[:], xall[:, xsl(0)])
            mm(0, s0[:], 0, FB)
            u_prev = xall[:, xsl(0)]

            for t in range(1, 14):
                un = up.tile([P, FB], mybir.dt.float32, name="u", tag="u")
                lif(un[:], u_prev, xall[:, xsl(t)])
                st = sp.tile([P, FB], mybir.dt.float8e4, name="s", tag="s")
                extract(t, st[:], un[:])
                mm(t, st[:], 0, FB)
                u_prev = un[:]
                if t == 7:
                    # plane A complete: pack + SWDGE store (hidden under
                    # the ongoing loads)
                    pack(0, FB, 127.5)
                    nc.gpsimd.dma_start(s_out[:, 0:FB], outp[:, 0:FB])

            # t14 in quarters (matches its quartered loads; lets each
            # t15 column slice start as soon as its t14 quarter is done)
            u14 = up.tile([P, FB], mybir.dt.float32, name="u", tag="u")
            s14 = sp.tile([P, FB], mybir.dt.float8e4, name="s", tag="s")
            for kq in range(4):
                sl = slice(kq * Q, (kq + 1) * Q)
                lif(u14[:, sl], u_prev[:, sl], xall[:, xsl(14, sl.start, sl.stop)])
                extract(14, s14[:, sl], u14[:, sl])
                mm(14, s14[:], sl.start, sl.stop)

            # t15 in quarters: lif -> extract -> matmul(stop) -> pack ->
            # store, stores on the by-now-idle sync HWDGE queue
            u15 = up.tile([P, FB], mybir.dt.float32, name="u", tag="u")
            s15 = sp.tile([P, FB], mybir.dt.float8e4, name="s", tag="s")
            for k in range(4):
                sl = slice(k * Q, (k + 1) * Q)
                lif(u15[:, sl], u14[:, sl], xall[:, xsl(15, sl.start, sl.stop)])
                extract(15, s15[:, sl], u15[:, sl])
                mm(15, s15[:], sl.start, sl.stop)
                pack(FB + sl.start, FB + sl.stop, 31.5)
                nc.sync.dma_start(
                    s_out[:, FB + sl.start:FB + sl.stop],
                    outp[:, FB + sl.start:FB + sl.stop],
                )


def _get_nc(repeat: int = 1):
    key = f"nc{repeat}"
    if key not in _CACHE:
        _CACHE[key] = _build(repeat)
    return _CACHE[key]


def _shard_input(x_seq: np.ndarray, i: int) -> np.ndarray:
    # [4, T, C, H, W] -> partition-major arena layout [P, T*B_LOC*F].
    xc = x_seq[i * B_LOC:(i + 1) * B_LOC].reshape(B_LOC, T, P, F)
    return np.ascontiguousarray(xc.transpose(2, 1, 0, 3).reshape(P, T * FB))


def _unshard_output(s_u8: np.ndarray) -> np.ndarray:
    # [P, 2*FB] u8 -> [B_LOC, T, C, H, W] f32 spikes.
    # Plane g in {0,1} holds bit j = spike at t = 8g + j.
    pl = s_u8.reshape(P, 2, B_LOC, F)
    bits = np.unpackbits(pl[..., None], axis=-1, bitorder="little")
    # dims (P, g, b, F, j) -> (b, g, j, P, F) -> [B_LOC, T, P, F]
    s = bits.transpose(2, 1, 4, 0, 3).reshape(B_LOC, T, P, F)
    return s.astype(np.float32).reshape(B_LOC, T, C, H, W)


def _run(x_seq: np.ndarray, trace: bool = False, repeat: int = 1):
    """Shard, execute on 8 cores, gather. Returns (output, BassKernelResults)."""
    nc = _get_nc(repeat)
    x_seq = np.ascontiguousarray(x_seq, dtype=np.float32)
    in_maps = [{"x": _shard_input(x_seq, i)} for i in range(N_CORES)]
    res = run_bass_kernel_spmd(
        nc, in_maps, core_ids=list(range(N_CORES)), trace=trace
    )
    out = np.concatenate(
        [_unshard_output(r["s"]) for r in res.results], axis=0
    )
    return out, res


def kernel(x_seq: np.ndarray) -> np.ndarray:
    out, _ = _run(x_seq, trace=False)
    return out
